# revision 36
# baseline (speedup 1.0000x reference)
"""MixHopNet GNN kernel for 8 Trainium2 NeuronCores (Bass/Tile SPMD).

Math (reference): GCN-normalized adjacency A = D^-1/2 (Adj + I) D^-1/2 over
N=50000 nodes / 800k random edges (+self loops), then
  x1 = A x ; x2 = A x1
  h  = relu([x w1_0 + b1_0, x1 w1_1 + b1_1, x2 w1_2 + b1_2])
  out = log_softmax([h w2_0 + b2_0, (A h) w2_1 + b2_1])

Distribution (graph/data parallel): nodes are packed into 456 blocks of 128
slots (the packing balances lo/hi-half in-edge counts per block, which
makes the cross-core chunk padding ~zero), 57 blocks per core.
Propagation for a dst block accumulates S_j^T @ V_j over edge chunks j of
128 edges in PSUM, where S_j is a 0/1 one-hot (edge -> dst slot)
PRECOMPUTED ON HOST, streamed from HBM as fp8_e4m3 (1.0 is exact), with an
identity prefix chunk per block that applies the reference's self loop.

P1 (x1 = A x): the table is known on host, so V is HOST-PREGATHERED and
streamed sequentially -- no dma_gather descriptors and no on-chip one-hot
builds.  P2/P3 V is dma_gather'd from the bf16 source-row table
(pre-scaled by dinv[src]); dinv[dst] is applied at PSUM eviction.
Gather calls are <=8 chunks (1024 descriptors -- the SWDGE ring size;
bigger calls crash) round-robin over the 4 SWDGE queues; each queue owns
a 4-DMA-engine subset, so the phase is paced by gather transfer time.
dma_gather needs int16 indices, so the table is split in two 29184-row
halves aligned with the node-id halves.

Cross-core: tables are replicated; x1 and z1 = dinv*(h @ w2_1) shards are
AllGather'd between propagations ((A h) w2_1 == A (h w2_1), so only 40
cols propagate in conv2).  Collectives must issue from GpSimd (issuing
from another engine wedges the device) and block the Pool engine until
completion, so they sit where Pool is idle anyway.  Dense work rides in
collective / gather-phase shadows: hT0 during AG1, hT1 at P2's start,
hT2 + z1 in the mid loop, out1 in AG2's shadow.

Softmax head avoids DVE tensor_scalar with per-partition scalars (~4.5us
each on HW!): t1n = -(logits+bias) via tensor_tensor, mxn = reduce_min,
ACT fuses shift+exp as Exp(-t1n + mxn) with accum_out, and the final
subtract is ACT Identity(-t1n + (mxn - lse)).  The Ln over row-sums runs
in 3 batches so most output stores overlap the P3 gather loop (each
Exp<->Ln function-table switch costs ~1.3us on ACT).
"""
import sys

sys.path.insert(0, "/opt/trn_rl_repo")

import numpy as np

import concourse.bass as bass  # noqa: F401
import concourse.bacc as bacc
import concourse.tile as tile
from concourse import mybir
from concourse.bass_utils import run_bass_kernel_spmd

import ml_dtypes

BF16 = ml_dtypes.bfloat16
FP8 = ml_dtypes.float8_e4m3

# ---- problem constants (hardcoded; kernel.py must be self-contained) ----
N = 50000
FIN = 128
H = 128
CO = 40
NCORES = 8
P = 128
NB = 456               # node blocks total
BPC = NB // NCORES     # 57 blocks per core
S = NB * P             # 58368 slots
NSH = BPC * P          # 7296 slots per core
HALF = S // 2          # 29184  (int16-addressable table half)
NHALF = N // 2         # node-id split matching the slot-half split
NQ = 4                 # SWDGE queues

LAST_RESULT = None     # BassKernelResults of the most recent run (for test.py)

_COMPILED = {}


# --------------------------------------------------------------------------
# host-side preprocessing
# --------------------------------------------------------------------------
def _pack_nodes(a, b):
    """Assign each node a slot: nodes [0,NHALF) -> blocks [0,NB/2), rest ->
    blocks [NB/2,NB).  Greedy 2D bin packing (descending total degree,
    minimize max(lo_load, hi_load)) balances both src-half in-edge counts."""
    node2slot = np.empty(N, np.int64)
    for hstart, hend, b0 in ((0, NHALF, 0), (NHALF, N, NB // 2)):
        nbins = NB // 2
        nodes = np.arange(hstart, hend)
        nodes = nodes[np.argsort(-(a[nodes] + b[nodes]), kind="stable")]
        lo_load = np.zeros(nbins, np.int64)
        hi_load = np.zeros(nbins, np.int64)
        cnt = np.zeros(nbins, np.int64)
        av = a[nodes]
        bv = b[nodes]
        for i in range(nodes.shape[0]):
            score = np.maximum(lo_load + av[i], hi_load + bv[i])
            score[cnt >= P] = 1 << 60
            blk = int(np.argmin(score))
            node2slot[nodes[i]] = (b0 + blk) * P + cnt[blk]
            cnt[blk] += 1
            lo_load[blk] += av[i]
            hi_load[blk] += bv[i]
    return node2slot


def _wrap_idx(flat):
    """[n*128] int16 -> [128, n*8] (16-partition wrap, replicated 8x)."""
    n = flat.shape[0] // 128
    arr = flat.reshape(n * 8, 16).T.copy()
    return np.tile(arr, (8, 1))


def _preprocess(x, edge_index, w1_0, b1_0, w1_1, b1_1, w1_2, b1_2,
                w2_0, b2_0, w2_1, b2_1):
    src = edge_index[0].astype(np.int64)
    dst = edge_index[1].astype(np.int64)
    E = src.shape[0]

    deg = (np.bincount(dst, minlength=N) + 1).astype(np.float32)  # +self loop
    dinv = (1.0 / np.sqrt(deg)).astype(np.float32)

    islo_n = src < NHALF
    a = np.bincount(dst[islo_n], minlength=N)
    b = np.bincount(dst[~islo_n], minlength=N)
    node2slot = _pack_nodes(a, b)

    # per-(old block, src half) edge counts, to sort/deal blocks to cores
    blk_old = node2slot[dst] >> 7
    grp_old = blk_old * 2 + (~islo_n).astype(np.int64)
    cnts = np.bincount(grp_old, minlength=2 * NB).reshape(NB, 2)
    tot = cnts.sum(axis=1)
    # block -> position: snake-deal blocks (sorted by total edges desc)
    # within each half so every core gets a similar block-size profile.
    order_bs = np.empty(NB, np.int64)
    blk_perm = np.empty(NB, np.int64)
    for half, coff in ((0, 0), (1, 4)):
        ids = np.arange(half * (NB // 2), (half + 1) * (NB // 2))
        ids = ids[np.argsort(-tot[ids], kind="stable")]
        percore = [[] for _ in range(4)]
        for i, bid in enumerate(ids):
            c = i % 4 if (i // 4) % 2 == 0 else 3 - i % 4
            percore[c].append(bid)
        for c in range(4):
            for j, bid in enumerate(percore[c]):
                pos = (coff + c) * BPC + j
                order_bs[pos] = bid
                blk_perm[bid] = pos
    slot_perm = (blk_perm[:, None] * P + np.arange(P)[None, :]).reshape(-1)
    node2slot = slot_perm[node2slot]
    cnts = cnts[order_bs]              # [position, half] edge counts

    dslot = node2slot[dst]
    nblk = dslot >> 7                  # block position (0..NB-1)
    d_in_blk = dslot & 127
    sslot = node2slot[src]
    islo = sslot < HALF                # == islo_n (halves preserved)
    sidx = np.where(islo, sslot, sslot - HALF).astype(np.int16)

    # chunk counts per (within-core position, half): max across cores
    ch = np.ceil(cnts / P).astype(np.int64).reshape(NCORES, BPC, 2)
    K_a = np.maximum(ch[:, :, 0].max(axis=0), 1)  # [BPC]
    K_b = np.maximum(ch[:, :, 1].max(axis=0), 1)
    Ktot = K_a + K_b
    TOTCH = int(Ktot.sum())
    PCH = TOTCH + BPC                  # + identity prefix chunk per block

    # canonical chunk order: per block [lo chunks, hi chunks]
    base_a = np.concatenate([[0], np.cumsum(Ktot)[:-1]])
    base_b = base_a + K_a

    # flatten edges into the padded chunk layout
    grp = nblk * 2 + (~islo).astype(np.int64)
    order = np.argsort(grp, kind="stable")
    gs = np.bincount(grp, minlength=2 * NB)
    starts = np.concatenate([[0], np.cumsum(gs)[:-1]])
    epos = np.arange(E) - starts[grp[order]]
    posc = np.arange(NB) % BPC
    gbase = np.stack([base_a[posc], base_b[posc]], axis=1)
    eslot = gbase[nblk[order], (~islo[order]).astype(np.int64)] * P + epos
    core_e = nblk[order] // BPC

    sidx_pad = np.zeros((NCORES, TOTCH * P), np.int16)
    sidx_pad[core_e, eslot] = sidx[order]
    gslot_pad = np.zeros((NCORES, TOTCH * P), np.int64)
    gslot_pad[core_e, eslot] = sslot[order]

    # one-hot stream, fp8, canonical order with identity prefix per block
    pbase = base_a + np.arange(BPC)
    ident8 = np.eye(P, dtype=FP8)
    one8 = np.float32(1.0).astype(FP8)
    oh8 = np.zeros((NCORES, PCH, P, P), FP8)
    ch_of_edge = eslot >> 7
    blk_of_ch = np.repeat(np.arange(BPC), Ktot)
    pch_of_edge = ch_of_edge + blk_of_ch[ch_of_edge] + 1
    oh8[core_e, pch_of_edge, eslot & 127, d_in_blk[order]] = one8
    oh8[:, pbase, :, :] = ident8[None, None]

    dinv_slot = np.zeros(S, np.float32)
    dinv_slot[node2slot] = dinv
    x_slot = np.zeros((S, FIN), np.float32)
    x_slot[node2slot] = x
    u0 = (x_slot * dinv_slot[:, None]).astype(BF16)

    brow_neg = np.tile(-np.concatenate([b2_0, b2_1])[None, :],
                       (P, 1)).astype(np.float32)
    w1s = np.concatenate([w1_0, w1_1, w1_2], axis=1).astype(BF16)
    b1m = np.stack([b1_0, b1_1, b1_2], axis=1).astype(np.float32)
    ident = np.eye(P, dtype=BF16)
    is_pref = np.zeros(PCH, bool)
    is_pref[pbase] = True

    in_maps = []
    for c in range(NCORES):
        rows = slice(c * NSH, (c + 1) * NSH)
        dm = dinv_slot[rows].reshape(BPC, P).T.copy()
        vg = np.empty((PCH, P, FIN), FP8)
        vg[~is_pref] = u0[gslot_pad[c]].reshape(TOTCH, P, FIN).astype(FP8)
        vg[pbase] = u0[rows].reshape(BPC, P, FIN).astype(FP8)
        vgc = np.ascontiguousarray(vg.transpose(1, 0, 2).reshape(P, PCH * FIN))
        ohc = np.ascontiguousarray(
            oh8[c].transpose(1, 0, 2).reshape(P, PCH * P))
        in_maps.append(dict(
            xT=np.ascontiguousarray(x_slot[rows].T).astype(BF16),
            idx=_wrap_idx(sidx_pad[c]),
            oh=ohc,
            vg=vgc,
            dinvc=dm,
            dinv2c=(dm * dm),
            ident=ident,
            w1s=w1s,
            b1m=b1m,
            w2a=np.asarray(w2_0, np.float32).astype(BF16),
            w2b=np.asarray(w2_1, np.float32).astype(BF16),
            brow_neg=brow_neg,
        ))
    return in_maps, node2slot, tuple(int(v) for v in K_a), \
        tuple(int(v) for v in K_b)


# --------------------------------------------------------------------------
# device program
# --------------------------------------------------------------------------
def _build(nc, K_a, K_b):
    dt = mybir.dt
    f32 = dt.float32
    bf16 = dt.bfloat16
    fp8 = dt.float8e4
    Ktot = [a + b for a, b in zip(K_a, K_b)]
    TOTCH = sum(Ktot)
    PCH = TOTCH + BPC
    KA_M, KB_M, KT_M = max(K_a), max(K_b), max(Ktot)
    cbase = [0]
    for k in Ktot:
        cbase.append(cbase[-1] + k)
    pbase = [cbase[b] + b for b in range(BPC)]   # stream position of prefix
    base_a = [cbase[b] for b in range(BPC)]
    base_b = [cbase[b] + K_a[b] for b in range(BPC)]

    xT = nc.dram_tensor("xT", [P, NSH], bf16, kind="ExternalInput").ap()
    idx = nc.dram_tensor("idx", [P, TOTCH * 8], dt.int16,
                         kind="ExternalInput").ap()
    ohd = nc.dram_tensor("oh", [P, PCH * P], fp8, kind="ExternalInput").ap()
    vgd = nc.dram_tensor("vg", [P, PCH * FIN], fp8, kind="ExternalInput").ap()
    dinvc = nc.dram_tensor("dinvc", [P, BPC], f32, kind="ExternalInput").ap()
    dinv2c = nc.dram_tensor("dinv2c", [P, BPC], f32, kind="ExternalInput").ap()
    identd = nc.dram_tensor("ident", [P, P], bf16, kind="ExternalInput").ap()
    w1s = nc.dram_tensor("w1s", [P, 3 * H], bf16, kind="ExternalInput").ap()
    b1m = nc.dram_tensor("b1m", [P, 3], f32, kind="ExternalInput").ap()
    w2a = nc.dram_tensor("w2a", [3 * H, CO], bf16, kind="ExternalInput").ap()
    w2b = nc.dram_tensor("w2b", [3 * H, CO], bf16, kind="ExternalInput").ap()
    brow_neg = nc.dram_tensor("brow_neg", [P, 2 * CO], f32,
                              kind="ExternalInput").ap()
    out = nc.dram_tensor("out", [NSH, 2 * CO], f32, kind="ExternalOutput").ap()

    rg = [list(range(NCORES))]

    with tile.TileContext(nc) as tc:
        with (
            tc.tile_pool(name="res", bufs=1) as res,
            tc.tile_pool(name="dram", bufs=1, space="DRAM") as dram,
        ):
            def load(name, src_ap, shape, dtype=f32, eng=nc.scalar):
                t = res.tile(shape, dtype, tag=name, name=name)
                eng.dma_start(out=t[:], in_=src_ap)
                return t

            # small tiles via ACT's HWDGE queue; idx via the mainline SWDGE
            # queue (Pool is idle through P1); vg streams own the SP queue.
            dinvc_t = load("dinvc", dinvc[:], [P, BPC])
            dinv2c_t = load("dinv2c", dinv2c[:], [P, BPC])
            ident_t = load("ident", identd[:], [P, P], bf16)
            w1_t = load("w1s", w1s[:], [P, 3 * H], bf16)
            b1_t = load("b1m", b1m[:], [P, 3])
            brow_t = load("brow_neg", brow_neg[:], [P, 2 * CO])
            w2a_t = [load(f"w2a{i}", w2a[i * H:(i + 1) * H, :], [P, CO], bf16)
                     for i in range(3)]
            w2b_t = [load(f"w2b{i}", w2b[i * H:(i + 1) * H, :], [P, CO], bf16)
                     for i in range(3)]
            idx_t = load("idx", idx[:], [P, TOTCH * 8], dt.int16,
                         eng=nc.gpsimd)

            x1T = res.tile([P, NSH], bf16, tag="x1T")
            x2T = res.tile([P, NSH], bf16, tag="x2T")
            hT = [res.tile([P, NSH], bf16, tag=f"hT{i}", name=f"hT{i}")
                  for i in range(3)]
            # work80[b]: [out1 | out2] logits, then t1n in place.
            work80 = res.tile([P, BPC * 2 * CO], f32, tag="work80")
            # first NRES blocks' one-hot chunks stay resident in SBUF across
            # all three propagations (saves ~10MB of repeated HBM streaming)
            BRES = 37          # blocks >= BRES keep one-hots resident
            RES0 = pbase[BRES]
            OHRES_CH = PCH - RES0
            ohres = load("ohres", ohd[:, RES0 * P:PCH * P],
                         [P, OHRES_CH * P], fp8)

            u1b = dram.tile([NSH, FIN], bf16, tag="u1b")
            u1f = dram.tile([S, FIN], bf16, tag="u1f", addr_space="Shared")
            uzb = dram.tile([NSH, P], bf16, tag="uzb")
            uzf = dram.tile([S, P], bf16, tag="uzf", addr_space="Shared")

            qn = [0]  # SWDGE queue round-robin

            def prop(tbl, own, width, evict, pools):
                """Per-(block, table-half) gather propagation (P2/P3)."""
                pwa, pwb, pp, ohp, sp = pools
                for b in range(BPC):
                    va = pwa.tile([P, KA_M, FIN], bf16, tag="va", name="va")
                    nc.gpsimd.dma_gather(
                        va[:, 0:K_a[b], :], tbl[0:HALF, :],
                        idx_t[:, base_a[b] * 8:(base_a[b] + K_a[b]) * 8],
                        num_idxs=K_a[b] * P, num_idxs_reg=K_a[b] * P,
                        elem_size=FIN, queue_num=qn[0])
                    qn[0] = (qn[0] + 1) % NQ
                    vb = pwb.tile([P, KB_M, FIN], bf16, tag="vb", name="vb")
                    nc.gpsimd.dma_gather(
                        vb[:, 0:K_b[b], :], tbl[HALF:S, :],
                        idx_t[:, base_b[b] * 8:(base_b[b] + K_b[b]) * 8],
                        num_idxs=K_b[b] * P, num_idxs_reg=K_b[b] * P,
                        elem_size=FIN, queue_num=qn[0])
                    qn[0] = (qn[0] + 1) % NQ
                    if b >= BRES:
                        o0 = pbase[b] - RES0
                        oht = ohres[:, o0 * P:(o0 + Ktot[b] + 1) * P]
                    else:
                        ot = ohp.tile([P, (KT_M + 1) * P], fp8, tag="oht")
                        oht = ot[:, 0:(Ktot[b] + 1) * P]
                        nc.scalar.dma_start(
                            out=oht,
                            in_=ohd[:, pbase[b] * P:
                                    (pbase[b] + Ktot[b] + 1) * P])
                    sblk = sp.tile([P, FIN], bf16, tag="sblk")
                    nc.sync.dma_start(out=sblk[:],
                                      in_=own[b * P:(b + 1) * P, :])
                    ps = pp.tile([P, width], f32, tag="agg")
                    nc.tensor.matmul(out=ps[:], lhsT=oht[:, 0:P],
                                     rhs=sblk[:, 0:width],
                                     start=True, stop=False)
                    for j in range(Ktot[b]):
                        srcv = (va[:, j, 0:width] if j < K_a[b]
                                else vb[:, j - K_a[b], 0:width])
                        nc.tensor.matmul(
                            out=ps[:],
                            lhsT=oht[:, (j + 1) * P:(j + 2) * P],
                            rhs=srcv,
                            start=False, stop=(j == Ktot[b] - 1))
                    evict(b, ps)

            # ================= P1: x1 = A x (streamed, no gathers) ==========
            with (
                tc.tile_pool(name="p1v", bufs=4) as vp,
                tc.tile_pool(name="p1p", bufs=4, space="PSUM") as pp,
                tc.tile_pool(name="p1o", bufs=4) as ohp,
                tc.tile_pool(name="p1e", bufs=3) as evp,
                tc.tile_pool(name="p1t", bufs=2, space="PSUM") as tpp,
            ):
                def evict1(b, ps):
                    x1t = evp.tile([P, P], bf16, tag="x1t")
                    nc.scalar.mul(x1t[:], ps[:], dinvc_t[:, b:b + 1])
                    u1t = evp.tile([P, P], bf16, tag="u1t")
                    nc.scalar.mul(u1t[:], ps[:], dinv2c_t[:, b:b + 1])
                    nc.sync.dma_start(out=u1b[b * P:(b + 1) * P, :],
                                      in_=u1t[:])
                    trp = tpp.tile([P, P], bf16, tag="trp")
                    nc.tensor.transpose(out=trp[:], in_=x1t[:],
                                        identity=ident_t[:])
                    nc.vector.tensor_copy(out=x1T[:, b * P:(b + 1) * P],
                                          in_=trp[:])

                for b in range(BPC):
                    nch = Ktot[b] + 1
                    vg = vp.tile([P, (KT_M + 1) * FIN], fp8, tag="vg")
                    nc.sync.dma_start(
                        out=vg[:, 0:nch * FIN],
                        in_=vgd[:, pbase[b] * FIN:(pbase[b] + nch) * FIN])
                    oht = ohp.tile([P, (KT_M + 1) * P], fp8, tag="oht")
                    nc.scalar.dma_start(
                        out=oht[:, 0:nch * P],
                        in_=ohd[:, pbase[b] * P:(pbase[b] + nch) * P])
                    ps = pp.tile([P, FIN], f32, tag="agg")
                    for j in range(nch):
                        nc.tensor.matmul(
                            out=ps[:],
                            lhsT=oht[:, j * P:(j + 1) * P],
                            rhs=vg[:, j * FIN:(j + 1) * FIN],
                            start=(j == 0), stop=(j == nch - 1))
                    evict1(b, ps)

            nc.gpsimd.collective_compute(
                "AllGather", mybir.AluOpType.bypass, replica_groups=rg,
                ins=[u1b.opt()], outs=[u1f.opt()])

            # hT[0] = relu(w1_0^T x^T + b1_0) only needs x -- run it in the
            # shadow of the AllGather.
            with (
                tc.tile_pool(name="d0x", bufs=3) as xsp0,
                tc.tile_pool(name="d0p", bufs=3, space="PSUM") as hpp0,
            ):
                for f0 in range(0, NSH, 512):
                    w = min(512, NSH - f0)
                    xt = xsp0.tile([P, 512], bf16, tag="xs0")
                    nc.sync.dma_start(out=xt[:, 0:w], in_=xT[:, f0:f0 + w])
                    ph = hpp0.tile([P, 512], f32, tag="hps0")
                    nc.tensor.matmul(out=ph[:, 0:w], lhsT=w1_t[:, 0:H],
                                     rhs=xt[:, 0:w], start=True, stop=True)
                    nc.scalar.activation(
                        out=hT[0][:, f0:f0 + w], in_=ph[:, 0:w],
                        func=mybir.ActivationFunctionType.Relu,
                        bias=b1_t[:, 0:1], scale=1.0)

            # ================= P2: x2 = A x1 =================
            # hT1 dense rides at P2's start (tensor is far from saturated
            # while the gather pipeline grinds).
            with (
                tc.tile_pool(name="p2wa", bufs=4) as pwa,
                tc.tile_pool(name="p2wb", bufs=4) as pwb,
                tc.tile_pool(name="p2p", bufs=4, space="PSUM") as pp,
                tc.tile_pool(name="p2o", bufs=8) as ohp,
                tc.tile_pool(name="p2s", bufs=8) as sp,
                tc.tile_pool(name="p2e", bufs=3) as evp,
                tc.tile_pool(name="p2t", bufs=2, space="PSUM") as tpp,
                tc.tile_pool(name="p2h", bufs=2, space="PSUM") as hpp,
            ):
                for f0 in range(0, NSH, 512):
                    w = min(512, NSH - f0)
                    ph = hpp.tile([P, 512], f32, tag="hps")
                    nc.tensor.matmul(out=ph[:, 0:w], lhsT=w1_t[:, H:2 * H],
                                     rhs=x1T[:, f0:f0 + w],
                                     start=True, stop=True)
                    nc.scalar.activation(
                        out=hT[1][:, f0:f0 + w], in_=ph[:, 0:w],
                        func=mybir.ActivationFunctionType.Relu,
                        bias=b1_t[:, 1:2], scale=1.0)

                def evict2(b, ps):
                    x2t = evp.tile([P, P], bf16, tag="x2t")
                    nc.scalar.mul(x2t[:], ps[:], dinvc_t[:, b:b + 1])
                    trp = tpp.tile([P, P], bf16, tag="trp2")
                    nc.tensor.transpose(out=trp[:], in_=x2t[:],
                                        identity=ident_t[:])
                    nc.vector.tensor_copy(out=x2T[:, b * P:(b + 1) * P],
                                          in_=trp[:])

                prop(u1f, u1b, FIN, evict2, (pwa, pwb, pp, ohp, sp))

            # ========== mid: hT2 dense + z1 fused, AllGather, out1 ==========
            with (
                tc.tile_pool(name="dps", bufs=3, space="PSUM") as hpp,
                tc.tile_pool(name="zps", bufs=2, space="PSUM") as zpp,
                tc.tile_pool(name="zev", bufs=3) as evp,
            ):
                for f0 in range(0, NSH, 512):
                    w = min(512, NSH - f0)
                    ph = hpp.tile([P, 512], f32, tag="hps")
                    nc.tensor.matmul(out=ph[:, 0:w], lhsT=w1_t[:, 2 * H:],
                                     rhs=x2T[:, f0:f0 + w],
                                     start=True, stop=True)
                    nc.scalar.activation(
                        out=hT[2][:, f0:f0 + w], in_=ph[:, 0:w],
                        func=mybir.ActivationFunctionType.Relu,
                        bias=b1_t[:, 2:3], scale=1.0)
                    for b in range(f0 // P, min((f0 + 512) // P, BPC)):
                        pz = zpp.tile([P, CO], f32, tag="pz")
                        for i in range(3):
                            nc.tensor.matmul(out=pz[:],
                                             lhsT=hT[i][:, b * P:(b + 1) * P],
                                             rhs=w2b_t[i][:], start=(i == 0),
                                             stop=(i == 2))
                        uzt = evp.tile([P, P], bf16, tag="uzt")
                        nc.scalar.mul(uzt[:, 0:CO], pz[:],
                                      dinvc_t[:, b:b + 1])
                        nc.sync.dma_start(out=uzb[b * P:(b + 1) * P, :],
                                          in_=uzt[:])

                nc.gpsimd.collective_compute(
                    "AllGather", mybir.AluOpType.bypass, replica_groups=rg,
                    ins=[uzb.opt()], outs=[uzf.opt()])

                # out1 = h w2_0 does not depend on the AllGather -- run it
                # in the collective's shadow.
                for b in range(BPC):
                    po = zpp.tile([P, CO], f32, tag="po")
                    for i in range(3):
                        nc.tensor.matmul(out=po[:],
                                         lhsT=hT[i][:, b * P:(b + 1) * P],
                                         rhs=w2a_t[i][:], start=(i == 0),
                                         stop=(i == 2))
                    nc.vector.tensor_copy(
                        out=work80[:, b * 2 * CO:b * 2 * CO + CO], in_=po[:])

            # ========== P3: out2 = dinv * A' z1, fused softmax head ==========
            mxn_all = res.tile([P, BPC], f32, tag="mxn_all")
            se_all = res.tile([P, BPC], f32, tag="se_all")
            lse = res.tile([P, BPC], f32, tag="lse")
            cb = res.tile([P, BPC], f32, tag="cb")
            LAG = 6
            BATCH = [(0, 28), (28, 51), (51, BPC)]  # lse/store batches
            with (
                tc.tile_pool(name="p3wa", bufs=4) as pwa,
                tc.tile_pool(name="p3wb", bufs=4) as pwb,
                tc.tile_pool(name="p3p", bufs=4, space="PSUM") as pp,
                tc.tile_pool(name="p3o", bufs=8) as ohp,
                tc.tile_pool(name="p3s", bufs=8) as sp,
                tc.tile_pool(name="p3f", bufs=4) as fp,
            ):
                def smax_exp(b):
                    ex = fp.tile([P, 2 * CO], f32, tag="ex")
                    nc.scalar.activation(
                        out=ex[:], in_=work80[:, b * 2 * CO:(b + 1) * 2 * CO],
                        func=mybir.ActivationFunctionType.Exp,
                        bias=mxn_all[:, b:b + 1], scale=-1.0,
                        accum_out=se_all[:, b:b + 1])

                def finish(lo, hi):
                    # r = t1 - mx - lse = -t1n + (mxn - lse)
                    nc.scalar.activation(
                        out=lse[:, lo:hi], in_=se_all[:, lo:hi],
                        func=mybir.ActivationFunctionType.Ln)
                    nc.vector.tensor_tensor(
                        out=cb[:, lo:hi], in0=mxn_all[:, lo:hi],
                        in1=lse[:, lo:hi], op=mybir.AluOpType.subtract)
                    for b in range(lo, hi):
                        r = fp.tile([P, 2 * CO], f32, tag="r")
                        nc.scalar.activation(
                            out=r[:],
                            in_=work80[:, b * 2 * CO:(b + 1) * 2 * CO],
                            func=mybir.ActivationFunctionType.Identity,
                            bias=cb[:, b:b + 1], scale=-1.0)
                        nc.sync.dma_start(out=out[b * P:(b + 1) * P, :],
                                          in_=r[:])

                def evict3(b, ps):
                    w80 = work80[:, b * 2 * CO:(b + 1) * 2 * CO]
                    nc.scalar.mul(
                        work80[:, b * 2 * CO + CO:(b + 1) * 2 * CO], ps[:],
                        dinvc_t[:, b:b + 1])
                    # t1n = -(logits + bias), in place over work80[b]
                    nc.vector.tensor_tensor(
                        out=w80, in0=brow_t[:], in1=w80,
                        op=mybir.AluOpType.subtract)
                    nc.vector.tensor_reduce(
                        out=mxn_all[:, b:b + 1], in_=w80,
                        axis=mybir.AxisListType.X, op=mybir.AluOpType.min)
                    if b >= LAG:
                        smax_exp(b - LAG)
                    for lo, hi in BATCH[:2]:
                        if b == hi + LAG - 1:
                            finish(lo, hi)

                prop(uzf, uzb, CO, evict3, (pwa, pwb, pp, ohp, sp))
                for b in range(BPC - LAG, BPC):
                    smax_exp(b)
                finish(*BATCH[2])


def _get_compiled(K_a, K_b):
    key = (K_a, K_b)
    if key not in _COMPILED:
        nc = bacc.Bacc("TRN2", target_bir_lowering=False, debug=False,
                       num_devices=NCORES, num_swdge_queues=NQ)
        _build(nc, K_a, K_b)
        nc.compile()
        _COMPILED[key] = nc
    return _COMPILED[key]


def kernel(**inputs):
    global LAST_RESULT
    args = {k: np.asarray(v) for k, v in inputs.items()}
    in_maps, node2slot, K_a, K_b = _preprocess(
        args["x"].astype(np.float32), args["edge_index"],
        args["w1_0"].astype(np.float32), args["b1_0"].astype(np.float32),
        args["w1_1"].astype(np.float32), args["b1_1"].astype(np.float32),
        args["w1_2"].astype(np.float32), args["b1_2"].astype(np.float32),
        args["w2_0"].astype(np.float32), args["b2_0"].astype(np.float32),
        args["w2_1"].astype(np.float32), args["b2_1"].astype(np.float32),
    )
    nc = _get_compiled(K_a, K_b)
    res = run_bass_kernel_spmd(nc, in_maps, list(range(NCORES)))
    LAST_RESULT = res
    out_slot = np.concatenate([res.results[c]["out"] for c in range(NCORES)],
                              axis=0)
    return out_slot[node2slot].astype(np.float32)


# revision 37
# speedup vs baseline: 1.0145x; 1.0145x over previous
"""MixHopNet GNN kernel for 8 Trainium2 NeuronCores (Bass/Tile SPMD).

Math (reference): GCN-normalized adjacency A = D^-1/2 (Adj + I) D^-1/2 over
N=50000 nodes / 800k random edges (+self loops), then
  x1 = A x ; x2 = A x1
  h  = relu([x w1_0 + b1_0, x1 w1_1 + b1_1, x2 w1_2 + b1_2])
  out = log_softmax([h w2_0 + b2_0, (A h) w2_1 + b2_1])

Distribution (graph/data parallel): nodes are packed into 456 blocks of 128
slots (the packing balances lo/hi-half in-edge counts per block, which
makes the cross-core chunk padding ~zero), 57 blocks per core.
Propagation for a dst block accumulates S_j^T @ V_j over edge chunks j of
128 edges in PSUM, where S_j is a 0/1 one-hot (edge -> dst slot)
PRECOMPUTED ON HOST, streamed from HBM as fp8_e4m3 (1.0 is exact), with an
identity prefix chunk per block that applies the reference's self loop.

P1 (x1 = A x): the table is known on host, so V is HOST-PREGATHERED and
streamed sequentially -- no dma_gather descriptors and no on-chip one-hot
builds.  P2/P3 V is dma_gather'd from the bf16 source-row table
(pre-scaled by dinv[src]); dinv[dst] is applied at PSUM eviction.
Gather calls are <=8 chunks (1024 descriptors -- the SWDGE ring size;
bigger calls crash) round-robin over the 4 SWDGE queues; each queue owns
a 4-DMA-engine subset, so the phase is paced by gather transfer time.
dma_gather needs int16 indices, so the table is split in two 29184-row
halves aligned with the node-id halves.

Cross-core: tables are replicated; x1 and z1 = dinv*(h @ w2_1) shards are
AllGather'd between propagations ((A h) w2_1 == A (h w2_1), so only 40
cols propagate in conv2).  Collectives must issue from GpSimd (issuing
from another engine wedges the device) and block the Pool engine until
completion, so they sit where Pool is idle anyway.  Dense work rides in
collective / gather-phase shadows: hT0 during AG1, hT1 at P2's start,
hT2 + z1 in the mid loop, out1 in AG2's shadow.

Softmax head avoids DVE tensor_scalar with per-partition scalars (~4.5us
each on HW!): t1n = -(logits+bias) via tensor_tensor, mxn = reduce_min,
ACT fuses shift+exp as Exp(-t1n + mxn) with accum_out, and the final
subtract is ACT Identity(-t1n + (mxn - lse)).  The Ln over row-sums runs
in 3 batches so most output stores overlap the P3 gather loop (each
Exp<->Ln function-table switch costs ~1.3us on ACT).
"""
import sys

sys.path.insert(0, "/opt/trn_rl_repo")

import numpy as np

import concourse.bass as bass  # noqa: F401
import concourse.bacc as bacc
import concourse.tile as tile
from concourse import mybir
from concourse.bass_utils import run_bass_kernel_spmd

import ml_dtypes

BF16 = ml_dtypes.bfloat16
FP8 = ml_dtypes.float8_e4m3

# ---- problem constants (hardcoded; kernel.py must be self-contained) ----
N = 50000
FIN = 128
H = 128
CO = 40
NCORES = 8
P = 128
NB = 456               # node blocks total
BPC = NB // NCORES     # 57 blocks per core
S = NB * P             # 58368 slots
NSH = BPC * P          # 7296 slots per core
HALF = S // 2          # 29184  (int16-addressable table half)
NHALF = N // 2         # node-id split matching the slot-half split
NQ = 4                 # SWDGE queues

LAST_RESULT = None     # BassKernelResults of the most recent run (for test.py)

_COMPILED = {}


# --------------------------------------------------------------------------
# host-side preprocessing
# --------------------------------------------------------------------------
def _pack_nodes(a, b):
    """Assign each node a slot: nodes [0,NHALF) -> blocks [0,NB/2), rest ->
    blocks [NB/2,NB).  Greedy 2D bin packing (descending total degree,
    minimize max(lo_load, hi_load)) balances both src-half in-edge counts."""
    node2slot = np.empty(N, np.int64)
    for hstart, hend, b0 in ((0, NHALF, 0), (NHALF, N, NB // 2)):
        nbins = NB // 2
        nodes = np.arange(hstart, hend)
        nodes = nodes[np.argsort(-(a[nodes] + b[nodes]), kind="stable")]
        lo_load = np.zeros(nbins, np.int64)
        hi_load = np.zeros(nbins, np.int64)
        cnt = np.zeros(nbins, np.int64)
        av = a[nodes]
        bv = b[nodes]
        for i in range(nodes.shape[0]):
            score = np.maximum(lo_load + av[i], hi_load + bv[i])
            score[cnt >= P] = 1 << 60
            blk = int(np.argmin(score))
            node2slot[nodes[i]] = (b0 + blk) * P + cnt[blk]
            cnt[blk] += 1
            lo_load[blk] += av[i]
            hi_load[blk] += bv[i]
    return node2slot


def _wrap_idx(flat):
    """[n*128] int16 -> [128, n*8] (16-partition wrap, replicated 8x)."""
    n = flat.shape[0] // 128
    arr = flat.reshape(n * 8, 16).T.copy()
    return np.tile(arr, (8, 1))


def _preprocess(x, edge_index, w1_0, b1_0, w1_1, b1_1, w1_2, b1_2,
                w2_0, b2_0, w2_1, b2_1):
    src = edge_index[0].astype(np.int64)
    dst = edge_index[1].astype(np.int64)
    E = src.shape[0]

    deg = (np.bincount(dst, minlength=N) + 1).astype(np.float32)  # +self loop
    dinv = (1.0 / np.sqrt(deg)).astype(np.float32)

    islo_n = src < NHALF
    a = np.bincount(dst[islo_n], minlength=N)
    b = np.bincount(dst[~islo_n], minlength=N)
    node2slot = _pack_nodes(a, b)

    # per-(old block, src half) edge counts, to sort/deal blocks to cores
    blk_old = node2slot[dst] >> 7
    grp_old = blk_old * 2 + (~islo_n).astype(np.int64)
    cnts = np.bincount(grp_old, minlength=2 * NB).reshape(NB, 2)
    tot = cnts.sum(axis=1)
    # block -> position: snake-deal blocks (sorted by total edges desc)
    # within each half so every core gets a similar block-size profile.
    order_bs = np.empty(NB, np.int64)
    blk_perm = np.empty(NB, np.int64)
    for half, coff in ((0, 0), (1, 4)):
        ids = np.arange(half * (NB // 2), (half + 1) * (NB // 2))
        ids = ids[np.argsort(-tot[ids], kind="stable")]
        percore = [[] for _ in range(4)]
        for i, bid in enumerate(ids):
            c = i % 4 if (i // 4) % 2 == 0 else 3 - i % 4
            percore[c].append(bid)
        for c in range(4):
            for j, bid in enumerate(percore[c]):
                pos = (coff + c) * BPC + j
                order_bs[pos] = bid
                blk_perm[bid] = pos
    slot_perm = (blk_perm[:, None] * P + np.arange(P)[None, :]).reshape(-1)
    node2slot = slot_perm[node2slot]
    cnts = cnts[order_bs]              # [position, half] edge counts

    dslot = node2slot[dst]
    nblk = dslot >> 7                  # block position (0..NB-1)
    d_in_blk = dslot & 127
    sslot = node2slot[src]
    islo = sslot < HALF                # == islo_n (halves preserved)
    sidx = np.where(islo, sslot, sslot - HALF).astype(np.int16)

    # chunk counts per (within-core position, half): max across cores
    ch = np.ceil(cnts / P).astype(np.int64).reshape(NCORES, BPC, 2)
    K_a = np.maximum(ch[:, :, 0].max(axis=0), 1)  # [BPC]
    K_b = np.maximum(ch[:, :, 1].max(axis=0), 1)
    Ktot = K_a + K_b
    TOTCH = int(Ktot.sum())
    PCH = TOTCH + BPC                  # + identity prefix chunk per block

    # canonical chunk order: per block [lo chunks, hi chunks]
    base_a = np.concatenate([[0], np.cumsum(Ktot)[:-1]])
    base_b = base_a + K_a

    # flatten edges into the padded chunk layout
    grp = nblk * 2 + (~islo).astype(np.int64)
    order = np.argsort(grp, kind="stable")
    gs = np.bincount(grp, minlength=2 * NB)
    starts = np.concatenate([[0], np.cumsum(gs)[:-1]])
    epos = np.arange(E) - starts[grp[order]]
    posc = np.arange(NB) % BPC
    gbase = np.stack([base_a[posc], base_b[posc]], axis=1)
    eslot = gbase[nblk[order], (~islo[order]).astype(np.int64)] * P + epos
    core_e = nblk[order] // BPC

    sidx_pad = np.zeros((NCORES, TOTCH * P), np.int16)
    sidx_pad[core_e, eslot] = sidx[order]
    gslot_pad = np.zeros((NCORES, TOTCH * P), np.int64)
    gslot_pad[core_e, eslot] = sslot[order]

    # one-hot stream, fp8, canonical order with identity prefix per block
    pbase = base_a + np.arange(BPC)
    ident8 = np.eye(P, dtype=FP8)
    one8 = np.float32(1.0).astype(FP8)
    oh8 = np.zeros((NCORES, PCH, P, P), FP8)
    ch_of_edge = eslot >> 7
    blk_of_ch = np.repeat(np.arange(BPC), Ktot)
    pch_of_edge = ch_of_edge + blk_of_ch[ch_of_edge] + 1
    oh8[core_e, pch_of_edge, eslot & 127, d_in_blk[order]] = one8
    oh8[:, pbase, :, :] = ident8[None, None]

    dinv_slot = np.zeros(S, np.float32)
    dinv_slot[node2slot] = dinv
    x_slot = np.zeros((S, FIN), np.float32)
    x_slot[node2slot] = x
    u0 = (x_slot * dinv_slot[:, None]).astype(BF16)

    brow_neg = np.tile(-np.concatenate([b2_0, b2_1])[None, :],
                       (P, 1)).astype(np.float32)
    w1s = np.concatenate([w1_0, w1_1, w1_2], axis=1).astype(BF16)
    b1m = np.stack([b1_0, b1_1, b1_2], axis=1).astype(np.float32)
    ident = np.eye(P, dtype=BF16)
    is_pref = np.zeros(PCH, bool)
    is_pref[pbase] = True

    in_maps = []
    for c in range(NCORES):
        rows = slice(c * NSH, (c + 1) * NSH)
        dm = dinv_slot[rows].reshape(BPC, P).T.copy()
        vg = np.empty((PCH, P, FIN), FP8)
        vg[~is_pref] = u0[gslot_pad[c]].reshape(TOTCH, P, FIN).astype(FP8)
        vg[pbase] = u0[rows].reshape(BPC, P, FIN).astype(FP8)
        vgc = np.ascontiguousarray(vg.transpose(1, 0, 2).reshape(P, PCH * FIN))
        ohc = np.ascontiguousarray(
            oh8[c].transpose(1, 0, 2).reshape(P, PCH * P))
        in_maps.append(dict(
            xT=np.ascontiguousarray(x_slot[rows].T).astype(BF16),
            idx=_wrap_idx(sidx_pad[c]),
            oh=ohc,
            vg=vgc,
            dinvc=dm,
            dinv2c=(dm * dm),
            ident=ident,
            w1s=w1s,
            b1m=b1m,
            w2a=np.asarray(w2_0, np.float32).astype(BF16),
            w2b=np.asarray(w2_1, np.float32).astype(BF16),
            brow_neg=brow_neg,
        ))
    return in_maps, node2slot, tuple(int(v) for v in K_a), \
        tuple(int(v) for v in K_b)


# --------------------------------------------------------------------------
# device program
# --------------------------------------------------------------------------
def _build(nc, K_a, K_b):
    dt = mybir.dt
    f32 = dt.float32
    bf16 = dt.bfloat16
    fp8 = dt.float8e4
    Ktot = [a + b for a, b in zip(K_a, K_b)]
    TOTCH = sum(Ktot)
    PCH = TOTCH + BPC
    KA_M, KB_M, KT_M = max(K_a), max(K_b), max(Ktot)
    cbase = [0]
    for k in Ktot:
        cbase.append(cbase[-1] + k)
    pbase = [cbase[b] + b for b in range(BPC)]   # stream position of prefix
    base_a = [cbase[b] for b in range(BPC)]
    base_b = [cbase[b] + K_a[b] for b in range(BPC)]

    xT = nc.dram_tensor("xT", [P, NSH], bf16, kind="ExternalInput").ap()
    idx = nc.dram_tensor("idx", [P, TOTCH * 8], dt.int16,
                         kind="ExternalInput").ap()
    ohd = nc.dram_tensor("oh", [P, PCH * P], fp8, kind="ExternalInput").ap()
    vgd = nc.dram_tensor("vg", [P, PCH * FIN], fp8, kind="ExternalInput").ap()
    dinvc = nc.dram_tensor("dinvc", [P, BPC], f32, kind="ExternalInput").ap()
    dinv2c = nc.dram_tensor("dinv2c", [P, BPC], f32, kind="ExternalInput").ap()
    identd = nc.dram_tensor("ident", [P, P], bf16, kind="ExternalInput").ap()
    w1s = nc.dram_tensor("w1s", [P, 3 * H], bf16, kind="ExternalInput").ap()
    b1m = nc.dram_tensor("b1m", [P, 3], f32, kind="ExternalInput").ap()
    w2a = nc.dram_tensor("w2a", [3 * H, CO], bf16, kind="ExternalInput").ap()
    w2b = nc.dram_tensor("w2b", [3 * H, CO], bf16, kind="ExternalInput").ap()
    brow_neg = nc.dram_tensor("brow_neg", [P, 2 * CO], f32,
                              kind="ExternalInput").ap()
    out = nc.dram_tensor("out", [NSH, 2 * CO], f32, kind="ExternalOutput").ap()

    rg = [list(range(NCORES))]

    with tile.TileContext(nc) as tc:
        with (
            tc.tile_pool(name="res", bufs=1) as res,
            tc.tile_pool(name="dram", bufs=1, space="DRAM") as dram,
        ):
            def load(name, src_ap, shape, dtype=f32, eng=nc.scalar):
                t = res.tile(shape, dtype, tag=name, name=name)
                eng.dma_start(out=t[:], in_=src_ap)
                return t

            # small tiles via ACT's HWDGE queue; idx via the mainline SWDGE
            # queue (Pool is idle through P1); vg streams own the SP queue.
            dinvc_t = load("dinvc", dinvc[:], [P, BPC])
            dinv2c_t = load("dinv2c", dinv2c[:], [P, BPC])
            ident_t = load("ident", identd[:], [P, P], bf16)
            w1_t = load("w1s", w1s[:], [P, 3 * H], bf16)
            b1_t = load("b1m", b1m[:], [P, 3])
            brow_t = load("brow_neg", brow_neg[:], [P, 2 * CO])
            w2a_t = [load(f"w2a{i}", w2a[i * H:(i + 1) * H, :], [P, CO], bf16)
                     for i in range(3)]
            w2b_t = [load(f"w2b{i}", w2b[i * H:(i + 1) * H, :], [P, CO], bf16)
                     for i in range(3)]
            idx_t = load("idx", idx[:], [P, TOTCH * 8], dt.int16,
                         eng=nc.gpsimd)

            x1T = res.tile([P, NSH], bf16, tag="x1T")
            x2T = res.tile([P, NSH], bf16, tag="x2T")
            hT = [res.tile([P, NSH], bf16, tag=f"hT{i}", name=f"hT{i}")
                  for i in range(3)]
            # work80[b]: [out1 | out2] logits, then t1n in place.
            work80 = res.tile([P, BPC * 2 * CO], f32, tag="work80")

            u1b = dram.tile([NSH, FIN], bf16, tag="u1b")
            u1f = dram.tile([S, FIN], bf16, tag="u1f", addr_space="Shared")
            uzb = dram.tile([NSH, P], bf16, tag="uzb")
            uzf = dram.tile([S, P], bf16, tag="uzf", addr_space="Shared")

            qn = [0]  # SWDGE queue round-robin

            def prop(tbl, own, width, evict, pools):
                """Per-(block, table-half) gather propagation (P2/P3)."""
                pwa, pwb, pp, ohp, sp = pools
                for b in range(BPC):
                    va = pwa.tile([P, KA_M, FIN], bf16, tag="va", name="va")
                    nc.gpsimd.dma_gather(
                        va[:, 0:K_a[b], :], tbl[0:HALF, :],
                        idx_t[:, base_a[b] * 8:(base_a[b] + K_a[b]) * 8],
                        num_idxs=K_a[b] * P, num_idxs_reg=K_a[b] * P,
                        elem_size=FIN, queue_num=qn[0])
                    qn[0] = (qn[0] + 1) % NQ
                    vb = pwb.tile([P, KB_M, FIN], bf16, tag="vb", name="vb")
                    nc.gpsimd.dma_gather(
                        vb[:, 0:K_b[b], :], tbl[HALF:S, :],
                        idx_t[:, base_b[b] * 8:(base_b[b] + K_b[b]) * 8],
                        num_idxs=K_b[b] * P, num_idxs_reg=K_b[b] * P,
                        elem_size=FIN, queue_num=qn[0])
                    qn[0] = (qn[0] + 1) % NQ
                    ot = ohp.tile([P, (KT_M + 1) * P], fp8, tag="oht")
                    oht = ot[:, 0:(Ktot[b] + 1) * P]
                    nc.scalar.dma_start(
                        out=oht,
                        in_=ohd[:, pbase[b] * P:
                                (pbase[b] + Ktot[b] + 1) * P])
                    sblk = sp.tile([P, FIN], bf16, tag="sblk")
                    nc.sync.dma_start(out=sblk[:],
                                      in_=own[b * P:(b + 1) * P, :])
                    ps = pp.tile([P, width], f32, tag="agg")
                    nc.tensor.matmul(out=ps[:], lhsT=oht[:, 0:P],
                                     rhs=sblk[:, 0:width],
                                     start=True, stop=False)
                    for j in range(Ktot[b]):
                        srcv = (va[:, j, 0:width] if j < K_a[b]
                                else vb[:, j - K_a[b], 0:width])
                        nc.tensor.matmul(
                            out=ps[:],
                            lhsT=oht[:, (j + 1) * P:(j + 2) * P],
                            rhs=srcv,
                            start=False, stop=(j == Ktot[b] - 1))
                    evict(b, ps)

            # ================= P1: x1 = A x (streamed, no gathers) ==========
            with (
                tc.tile_pool(name="p1v", bufs=4) as vp,
                tc.tile_pool(name="p1p", bufs=4, space="PSUM") as pp,
                tc.tile_pool(name="p1o", bufs=4) as ohp,
                tc.tile_pool(name="p1e", bufs=3) as evp,
                tc.tile_pool(name="p1t", bufs=2, space="PSUM") as tpp,
            ):
                def evict1(b, ps):
                    x1t = evp.tile([P, P], bf16, tag="x1t")
                    nc.scalar.mul(x1t[:], ps[:], dinvc_t[:, b:b + 1])
                    u1t = evp.tile([P, P], bf16, tag="u1t")
                    nc.scalar.mul(u1t[:], ps[:], dinv2c_t[:, b:b + 1])
                    nc.sync.dma_start(out=u1b[b * P:(b + 1) * P, :],
                                      in_=u1t[:])
                    trp = tpp.tile([P, P], bf16, tag="trp")
                    nc.tensor.transpose(out=trp[:], in_=x1t[:],
                                        identity=ident_t[:])
                    nc.vector.tensor_copy(out=x1T[:, b * P:(b + 1) * P],
                                          in_=trp[:])

                for b in range(BPC):
                    nch = Ktot[b] + 1
                    vg = vp.tile([P, (KT_M + 1) * FIN], fp8, tag="vg")
                    nc.sync.dma_start(
                        out=vg[:, 0:nch * FIN],
                        in_=vgd[:, pbase[b] * FIN:(pbase[b] + nch) * FIN])
                    oht = ohp.tile([P, (KT_M + 1) * P], fp8, tag="oht")
                    nc.scalar.dma_start(
                        out=oht[:, 0:nch * P],
                        in_=ohd[:, pbase[b] * P:(pbase[b] + nch) * P])
                    ps = pp.tile([P, FIN], f32, tag="agg")
                    for j in range(nch):
                        nc.tensor.matmul(
                            out=ps[:],
                            lhsT=oht[:, j * P:(j + 1) * P],
                            rhs=vg[:, j * FIN:(j + 1) * FIN],
                            start=(j == 0), stop=(j == nch - 1))
                    evict1(b, ps)

            nc.gpsimd.collective_compute(
                "AllGather", mybir.AluOpType.bypass, replica_groups=rg,
                ins=[u1b.opt()], outs=[u1f.opt()])

            # hT[0] = relu(w1_0^T x^T + b1_0) only needs x -- run it in the
            # shadow of the AllGather.
            with (
                tc.tile_pool(name="d0x", bufs=3) as xsp0,
                tc.tile_pool(name="d0p", bufs=3, space="PSUM") as hpp0,
            ):
                for f0 in range(0, NSH, 512):
                    w = min(512, NSH - f0)
                    xt = xsp0.tile([P, 512], bf16, tag="xs0")
                    nc.sync.dma_start(out=xt[:, 0:w], in_=xT[:, f0:f0 + w])
                    ph = hpp0.tile([P, 512], f32, tag="hps0")
                    nc.tensor.matmul(out=ph[:, 0:w], lhsT=w1_t[:, 0:H],
                                     rhs=xt[:, 0:w], start=True, stop=True)
                    nc.scalar.activation(
                        out=hT[0][:, f0:f0 + w], in_=ph[:, 0:w],
                        func=mybir.ActivationFunctionType.Relu,
                        bias=b1_t[:, 0:1], scale=1.0)

            # ================= P2: x2 = A x1 =================
            # hT1 dense rides at P2's start (tensor is far from saturated
            # while the gather pipeline grinds).
            with (
                tc.tile_pool(name="p2wa", bufs=4) as pwa,
                tc.tile_pool(name="p2wb", bufs=4) as pwb,
                tc.tile_pool(name="p2p", bufs=4, space="PSUM") as pp,
                tc.tile_pool(name="p2o", bufs=8) as ohp,
                tc.tile_pool(name="p2s", bufs=8) as sp,
                tc.tile_pool(name="p2e", bufs=3) as evp,
                tc.tile_pool(name="p2t", bufs=2, space="PSUM") as tpp,
                tc.tile_pool(name="p2h", bufs=2, space="PSUM") as hpp,
            ):
                for f0 in range(0, NSH, 512):
                    w = min(512, NSH - f0)
                    ph = hpp.tile([P, 512], f32, tag="hps")
                    nc.tensor.matmul(out=ph[:, 0:w], lhsT=w1_t[:, H:2 * H],
                                     rhs=x1T[:, f0:f0 + w],
                                     start=True, stop=True)
                    nc.scalar.activation(
                        out=hT[1][:, f0:f0 + w], in_=ph[:, 0:w],
                        func=mybir.ActivationFunctionType.Relu,
                        bias=b1_t[:, 1:2], scale=1.0)

                def evict2(b, ps):
                    x2t = evp.tile([P, P], bf16, tag="x2t")
                    nc.scalar.mul(x2t[:], ps[:], dinvc_t[:, b:b + 1])
                    trp = tpp.tile([P, P], bf16, tag="trp2")
                    nc.tensor.transpose(out=trp[:], in_=x2t[:],
                                        identity=ident_t[:])
                    nc.vector.tensor_copy(out=x2T[:, b * P:(b + 1) * P],
                                          in_=trp[:])

                prop(u1f, u1b, FIN, evict2, (pwa, pwb, pp, ohp, sp))

            # ========== mid: hT2 dense + z1 fused, AllGather, out1 ==========
            with (
                tc.tile_pool(name="dps", bufs=3, space="PSUM") as hpp,
                tc.tile_pool(name="zps", bufs=2, space="PSUM") as zpp,
                tc.tile_pool(name="zev", bufs=3) as evp,
            ):
                for f0 in range(0, NSH, 512):
                    w = min(512, NSH - f0)
                    ph = hpp.tile([P, 512], f32, tag="hps")
                    nc.tensor.matmul(out=ph[:, 0:w], lhsT=w1_t[:, 2 * H:],
                                     rhs=x2T[:, f0:f0 + w],
                                     start=True, stop=True)
                    nc.scalar.activation(
                        out=hT[2][:, f0:f0 + w], in_=ph[:, 0:w],
                        func=mybir.ActivationFunctionType.Relu,
                        bias=b1_t[:, 2:3], scale=1.0)
                    for b in range(f0 // P, min((f0 + 512) // P, BPC)):
                        pz = zpp.tile([P, CO], f32, tag="pz")
                        for i in range(3):
                            nc.tensor.matmul(out=pz[:],
                                             lhsT=hT[i][:, b * P:(b + 1) * P],
                                             rhs=w2b_t[i][:], start=(i == 0),
                                             stop=(i == 2))
                        uzt = evp.tile([P, P], bf16, tag="uzt")
                        nc.scalar.mul(uzt[:, 0:CO], pz[:],
                                      dinvc_t[:, b:b + 1])
                        nc.sync.dma_start(out=uzb[b * P:(b + 1) * P, :],
                                          in_=uzt[:])

                nc.gpsimd.collective_compute(
                    "AllGather", mybir.AluOpType.bypass, replica_groups=rg,
                    ins=[uzb.opt()], outs=[uzf.opt()])

                # out1 = h w2_0 does not depend on the AllGather -- run it
                # in the collective's shadow.
                for b in range(BPC):
                    po = zpp.tile([P, CO], f32, tag="po")
                    for i in range(3):
                        nc.tensor.matmul(out=po[:],
                                         lhsT=hT[i][:, b * P:(b + 1) * P],
                                         rhs=w2a_t[i][:], start=(i == 0),
                                         stop=(i == 2))
                    nc.vector.tensor_copy(
                        out=work80[:, b * 2 * CO:b * 2 * CO + CO], in_=po[:])

            # ========== P3: out2 = dinv * A' z1, fused softmax head ==========
            mxn_all = res.tile([P, BPC], f32, tag="mxn_all")
            se_all = res.tile([P, BPC], f32, tag="se_all")
            lse = res.tile([P, BPC], f32, tag="lse")
            cb = res.tile([P, BPC], f32, tag="cb")
            LAG = 8
            BATCH = [(0, 24), (24, 48), (48, BPC)]  # lse/store batches
            with (
                tc.tile_pool(name="p3wa", bufs=4) as pwa,
                tc.tile_pool(name="p3wb", bufs=4) as pwb,
                tc.tile_pool(name="p3p", bufs=4, space="PSUM") as pp,
                tc.tile_pool(name="p3o", bufs=8) as ohp,
                tc.tile_pool(name="p3s", bufs=8) as sp,
                tc.tile_pool(name="p3f", bufs=4) as fp,
            ):
                def smax_exp(b):
                    ex = fp.tile([P, 2 * CO], f32, tag="ex")
                    nc.scalar.activation(
                        out=ex[:], in_=work80[:, b * 2 * CO:(b + 1) * 2 * CO],
                        func=mybir.ActivationFunctionType.Exp,
                        bias=mxn_all[:, b:b + 1], scale=-1.0,
                        accum_out=se_all[:, b:b + 1])

                def finish(lo, hi):
                    # r = t1 - mx - lse = -t1n + (mxn - lse)
                    nc.scalar.activation(
                        out=lse[:, lo:hi], in_=se_all[:, lo:hi],
                        func=mybir.ActivationFunctionType.Ln)
                    nc.vector.tensor_tensor(
                        out=cb[:, lo:hi], in0=mxn_all[:, lo:hi],
                        in1=lse[:, lo:hi], op=mybir.AluOpType.subtract)
                    for b in range(lo, hi):
                        r = fp.tile([P, 2 * CO], f32, tag="r")
                        nc.scalar.activation(
                            out=r[:],
                            in_=work80[:, b * 2 * CO:(b + 1) * 2 * CO],
                            func=mybir.ActivationFunctionType.Identity,
                            bias=cb[:, b:b + 1], scale=-1.0)
                        nc.sync.dma_start(out=out[b * P:(b + 1) * P, :],
                                          in_=r[:])

                def evict3(b, ps):
                    w80 = work80[:, b * 2 * CO:(b + 1) * 2 * CO]
                    nc.scalar.mul(
                        work80[:, b * 2 * CO + CO:(b + 1) * 2 * CO], ps[:],
                        dinvc_t[:, b:b + 1])
                    # t1n = -(logits + bias), in place over work80[b]
                    nc.vector.tensor_tensor(
                        out=w80, in0=brow_t[:], in1=w80,
                        op=mybir.AluOpType.subtract)
                    nc.vector.tensor_reduce(
                        out=mxn_all[:, b:b + 1], in_=w80,
                        axis=mybir.AxisListType.X, op=mybir.AluOpType.min)
                    if b >= LAG:
                        smax_exp(b - LAG)
                    for lo, hi in BATCH[:2]:
                        if b == hi + LAG - 1:
                            finish(lo, hi)

                prop(uzf, uzb, CO, evict3, (pwa, pwb, pp, ohp, sp))
                for b in range(BPC - LAG, BPC):
                    smax_exp(b)
                finish(*BATCH[2])


def _get_compiled(K_a, K_b):
    key = (K_a, K_b)
    if key not in _COMPILED:
        nc = bacc.Bacc("TRN2", target_bir_lowering=False, debug=False,
                       num_devices=NCORES, num_swdge_queues=NQ)
        _build(nc, K_a, K_b)
        nc.compile()
        _COMPILED[key] = nc
    return _COMPILED[key]


def kernel(**inputs):
    global LAST_RESULT
    args = {k: np.asarray(v) for k, v in inputs.items()}
    in_maps, node2slot, K_a, K_b = _preprocess(
        args["x"].astype(np.float32), args["edge_index"],
        args["w1_0"].astype(np.float32), args["b1_0"].astype(np.float32),
        args["w1_1"].astype(np.float32), args["b1_1"].astype(np.float32),
        args["w1_2"].astype(np.float32), args["b1_2"].astype(np.float32),
        args["w2_0"].astype(np.float32), args["b2_0"].astype(np.float32),
        args["w2_1"].astype(np.float32), args["b2_1"].astype(np.float32),
    )
    nc = _get_compiled(K_a, K_b)
    res = run_bass_kernel_spmd(nc, in_maps, list(range(NCORES)))
    LAST_RESULT = res
    out_slot = np.concatenate([res.results[c]["out"] for c in range(NCORES)],
                              axis=0)
    return out_slot[node2slot].astype(np.float32)
